# revision 54
# baseline (speedup 1.0000x reference)
"""Trainium2 Bass kernel for nn_Attention_82051055223090.

ViT-style multi-head attention with RoPE on non-CLS tokens:
  qkv = x @ w_qkv + b_qkv ; rope(q,k) ; softmax(q k^T / sqrt(D)) v ; proj.

Strategy: pure data-parallel over batch (B=32 -> 4 per core x 8 cores), no
collectives.  Matmul operands are bf16 (full PE rate + fast weight load);
accumulation is fp32 in PSUM, softmax in fp32.  All layout transforms
(x transpose, rope-table transpose/swap, bias broadcast) happen host-side in
numpy during input marshalling so every device DMA is contiguous.

Per-core dataflow (per batch element):
  xT[c,t]   <- contiguous bf16 DMA (pre-transposed on host)   [768, 577]
  qk        = w_qkv[:, :1536]^T @ x^T; the bias add and the rope table
            multiplies are fused scalar_tensor_tensor ops reading PSUM:
            ut = (qk+b)*sinS, qc = (qk+b)*cosT.  The rotate-half partition
            swap runs as 4 batched SBUF->SBUF DMAs per 6-tile group
            (q group, k group), then qf = qc + us in one add per tile group.
  v         = x @ w_qkv[:, 1536:]  (+b), packed per head pair as
            [v_even | ones | v_odd]; the ones block makes the AV matmul
            also emit the softmax denominator broadcast over 64 rows
  scores    per pair: the two heads' score matmuls use K=64 row groups
            (0:64 / 64:128) and are issued back-to-back so they overlap on
            the PE (array row tiling).  Queries are processed in two
            289/288-token phases so PSUM stays within 8 banks; AV matmuls
            lag the score wave by two steps, and each pair's final AV+norm
            is deferred behind the next pair's first wave, so exp (ACT)
            latency never stalls the PE.
  outT|den  = [v|ones]-as-lhsT @ expT   (PSUM fp32, accumulated over j)
  normOut   : av banks are copied to SBUF on the scalar engine (fast bank
            release), then normalized via reciprocal_approx_fast (which
            only works on partitions 0:64) + one gpsimd partition-move DMA
            per phase-half
  out       : transposed projection out[c_out, tokens] = w_projT-as-lhsT @
            normOut -- full 128-row c-tiles, 577-col free dim, per-partition
            bias; bf16 DMA out in [C, N] layout, host untransposes/upcasts.
            The v/proj biases are folded together host-side (softmax weights
            sum to 1), so b_proj_eff = b_v @ W_p + b_p.
"""

import numpy as np

B, N, C, H, D = 32, 577, 768, 12, 64
NCORES = 8
NB = B // NCORES          # batches per core
P = 128
KT = C // P               # 6 contraction chunks of 128
NPAIR = H // 2            # 6 head pairs
TOK = [(i * P, min(P, N - i * P)) for i in range((N + P - 1) // P)]  # key tiles
NA = 289                  # query chunk A = [0:289]
NBW = 288                 # query chunk B = [289:577]

_cache = {}


def _build():
    from contextlib import ExitStack

    import concourse.tile as tile
    from concourse import bacc, mybir
    from concourse.ap import AP

    f32 = mybir.dt.float32
    bf16 = mybir.dt.bfloat16
    AF = mybir.ActivationFunctionType
    OP = mybir.AluOpType

    nc = bacc.Bacc("TRN2", debug=False, enable_partition_id=False)

    xt_d = nc.dram_tensor("xt", [NB, C, N], bf16, kind="ExternalInput").ap()
    wqkv_d = nc.dram_tensor("w_qkv", [C, 3 * C], bf16, kind="ExternalInput").ap()
    wproj_d = nc.dram_tensor("w_proj", [C, C], bf16, kind="ExternalInput").ap()
    bqk_d = nc.dram_tensor("bqk2", [P, 18], f32, kind="ExternalInput").ap()
    bpb_d = nc.dram_tensor("bpb", [P, KT], f32, kind="ExternalInput").ap()
    cost_d = nc.dram_tensor("cost", [64, N], bf16, kind="ExternalInput").ap()
    sins_d = nc.dram_tensor("sins", [64, N], bf16, kind="ExternalInput").ap()
    out_d = nc.dram_tensor("out", [NB, C, N], bf16, kind="ExternalOutput").ap()

    def ap3(base_ap, part_off, elem_off, dims):
        """Raw AP on the same tensor: partition slice + multi-dim free dims."""
        rowstr = base_ap.ap[0][0]
        return AP(
            base_ap.tensor,
            base_ap.offset + part_off * rowstr + elem_off,
            [[rowstr, dims[0]]] + [list(d) for d in dims[1:]],
        )

    with tile.TileContext(nc) as tc, ExitStack() as ctx:
        const = ctx.enter_context(tc.tile_pool(name="const", bufs=1))
        ps = ctx.enter_context(tc.tile_pool(name="ps", bufs=2, space="PSUM"))
        avp = ctx.enter_context(tc.tile_pool(name="avp", bufs=2, space="PSUM"))
        scp = ctx.enter_context(tc.tile_pool(name="scp", bufs=2, space="PSUM"))
        sb = ctx.enter_context(tc.tile_pool(name="sb", bufs=1))

        # ---- constants (all pre-formatted on host, contiguous DMAs) ----
        def emit_x(b):
            xts = []
            for k in range(KT):
                xt = sb.tile([P, N], bf16, tag="xt", bufs=12, name=f"xt{b}_{k}")
                nc.sync.dma_start(xt, xt_d[b, k * P:(k + 1) * P, :])
                xts.append(xt)
            return xts

        # interleave w_qkv chunks with batch 0's xt chunks so the first
        # matmul (needs w0 + xt0 only) starts as early as possible
        w_sb = []
        xts0 = []
        for k in range(KT):
            w = const.tile([P, 3 * C], bf16, tag=f"w{k}", name=f"w{k}")
            nc.sync.dma_start(w, wqkv_d[k * P:(k + 1) * P, :])
            w_sb.append(w)
            xt = sb.tile([P, N], bf16, tag="xt", bufs=12, name=f"xt0_{k}")
            # split in two so early token tiles unblock at half the latency
            nc.sync.dma_start(xt[:, 0:NA], xt_d[0, k * P:(k + 1) * P, 0:NA])
            nc.sync.dma_start(xt[:, NA:N], xt_d[0, k * P:(k + 1) * P, NA:N])
            xts0.append(xt)

        cosT = const.tile([P, N], bf16, tag="cosT", name="cosT")
        sinS = const.tile([P, N], bf16, tag="sinS", name="sinS")
        for g in range(2):  # duplicate across the two 64-partition groups
            nc.sync.dma_start(cosT[g * 64:g * 64 + 64, :], cost_d)
            nc.sync.dma_start(sinS[g * 64:g * 64 + 64, :], sins_d)

        bqk = const.tile([P, 18], f32, tag="bqk", name="bqk")
        nc.sync.dma_start(bqk, bqk_d)

        # proj weights load after batch 0's xt (not needed until first proj)
        wp_sb = [const.tile([P, C], bf16, tag=f"wp{k}", name=f"wp{k}")
                 for k in range(KT)]
        bpB = const.tile([P, KT], f32, tag="bpB", name="bpB")

        def emit_proj_consts():
            for k in range(KT):
                nc.sync.dma_start(wp_sb[k], wproj_d[k * P:(k + 1) * P, :])
            nc.sync.dma_start(bpB, bpb_d)

        def emit_front(b, xts):
            vts = []
            for it, (ts, tsz) in enumerate(TOK):
                vt = sb.tile([P, NPAIR * 192], bf16, tag="v", bufs=11, name=f"v{b}_{it}")
                for half in range(2):
                    pv = ps.tile([P, 512], f32, tag="ps", name=f"pv{b}_{it}_{half}")
                    c0 = 2 * C + half * 384
                    for k in range(KT):
                        nc.tensor.matmul(
                            pv[0:tsz, 0:384],
                            xts[k][:, ts:ts + tsz],
                            w_sb[k][:, c0:c0 + 384],
                            start=(k == 0), stop=(k == KT - 1))
                    po = 0 if half == 0 else 576
                    dst = ap3(vt[:], 0, po, [tsz, (192, 3), (128, 2), (1, 64)])
                    src_ = pv[0:tsz, 0:384].rearrange("p (a c d) -> p a c d", a=3, c=2)
                    # v bias is folded through the projection into bpb
                    # (softmax weights sum to 1), so this is a pure copy
                    nc.vector.tensor_scalar(dst, src_, 0.0, None, OP.add)
                ones = ap3(vt[:], 0, 64, [tsz, (192, NPAIR), (1, 64)])
                nc.vector.memset(ones, 1.0)
                vts.append(vt)
            return vts

        def emit_qk_warmup(xts, ut_all, qc_all):
            """First 4 qk tiles of batch 0, k-major across 8 borrowed PSUM
            banks, so the PE keeps pace with the initial w/xt HBM stream."""
            pa0 = ps.tile([P, 512], f32, tag="ps", name="wq_a0")
            pb0 = ps.tile([P, 512], f32, tag="ps", name="wq_b0")
            pa1 = avp.tile([P, 512], f32, tag="av", name="wq_a1")
            pb1 = avp.tile([P, 512], f32, tag="av", name="wq_b1")
            s2 = scp.tile([P, 1024], f32, tag="sc", name="wq_2")
            s3 = scp.tile([P, 1024], f32, tag="sc", name="wq_3")
            groups = [
                (pa0[:, 0:NA], pb0[:, 0:NBW]),
                (pa1[:, 0:NA], pb1[:, 0:NBW]),
                (s2[:, 0:NA], s2[:, 512:512 + NBW]),
                (s3[:, 0:NA], s3[:, 512:512 + NBW]),
            ]
            for k in range(KT):
                for m in range(4):
                    oA, oB = groups[m]
                    nc.tensor.matmul(
                        oA, w_sb[k][:, m * P:(m + 1) * P], xts[k][:, 0:NA],
                        start=(k == 0), stop=(k == KT - 1),
                        skip_group_check=True)
                    nc.tensor.matmul(
                        oB, w_sb[k][:, m * P:(m + 1) * P], xts[k][:, NA:N],
                        start=(k == 0), stop=(k == KT - 1),
                        skip_group_check=True)
            stt = nc.vector.scalar_tensor_tensor

            def drain(m):
                oA, oB = groups[m]
                stt(ut_all[:, m, 0:NA], oA, bqk[:, m:m + 1],
                    sinS[:, 0:NA], OP.add, OP.mult)
                stt(qc_all[:, m, 0:NA], oA, bqk[:, m:m + 1],
                    cosT[:, 0:NA], OP.add, OP.mult)
                stt(ut_all[:, m, NA:N], oB, bqk[:, m:m + 1],
                    sinS[:, NA:N], OP.add, OP.mult)
                stt(qc_all[:, m, NA:N], oB, bqk[:, m:m + 1],
                    cosT[:, NA:N], OP.add, OP.mult)

            # drain only m0 now (frees the ps-pool banks tile m=4 needs);
            # m1-m3 drains are deferred so they don't clog the DVE queue
            drain(0)
            return [lambda m=m: drain(m) for m in range(1, 4)]

        def emit_qk_tile(b, m, xts, ut_all, qc_all, pool=None, ptag="ps"):
            """qk tile m: matmul into PSUM, then fused (psum+bias)*table ops."""
            pool = pool or ps
            pA = pool.tile([P, 512], f32, tag=ptag, name=f"pqa{b}_{m}")
            pB = pool.tile([P, 512], f32, tag=ptag, name=f"pqb{b}_{m}")
            for k in range(KT):
                nc.tensor.matmul(
                    pA[:, 0:NA], w_sb[k][:, m * P:(m + 1) * P],
                    xts[k][:, 0:NA],
                    start=(k == 0), stop=(k == KT - 1))
            for k in range(KT):
                nc.tensor.matmul(
                    pB[:, 0:NBW], w_sb[k][:, m * P:(m + 1) * P],
                    xts[k][:, NA:N],
                    start=(k == 0), stop=(k == KT - 1))
            stt = nc.vector.scalar_tensor_tensor
            stt(ut_all[:, m, 0:NA], pA[:, 0:NA], bqk[:, m:m + 1],
                sinS[:, 0:NA], OP.add, OP.mult)
            stt(qc_all[:, m, 0:NA], pA[:, 0:NA], bqk[:, m:m + 1],
                cosT[:, 0:NA], OP.add, OP.mult)
            stt(ut_all[:, m, NA:N], pB[:, 0:NBW], bqk[:, m:m + 1],
                sinS[:, NA:N], OP.add, OP.mult)
            stt(qc_all[:, m, NA:N], pB[:, 0:NBW], bqk[:, m:m + 1],
                cosT[:, NA:N], OP.add, OP.mult)

        def emit_swap(b, g, ut_all, us_all):
            """Partition swap for 6-tile group g: 4 batched SBUF->SBUF DMAs."""
            m0, m1 = g * 6, (g + 1) * 6
            for blk in range(4):
                o0, i0 = blk * 32, (blk ^ 1) * 32
                nc.gpsimd.dma_start(
                    us_all[o0:o0 + 32, m0:m1, :], ut_all[i0:i0 + 32, m0:m1, :])

        carry = [None]   # deferred last-AV+norm of the previous attention
        carry_b = [None]

        def emit_attention(b, pair, qc_all, vts, no_sb):
            no_t = sb.tile([P, N], bf16, tag="no", bufs=9, name=f"no{b}_{pair}")
            qt = qc_all[:, pair, :]
            kt = qc_all[:, 6 + pair, :]
            waves = [(0, 0, NA), (1, NA, NBW)]  # (phase, q0, qw)
            # wave list: phase x key-tile; AV lags the score wave by one step
            wl = [(ph, q0, qw, jc) for (ph, q0, qw) in waves
                  for jc in range(len(TOK))]
            avs = {}
            ets = {}

            def emit_av(idx):
                ph, q0, qw, jc = wl[idx]
                js, jsz = TOK[jc]
                av0, av1 = avs[ph]
                et = ets.pop(idx)
                v0 = vts[jc][0:jsz, pair * 192:pair * 192 + 128]
                v1 = vts[jc][0:jsz, pair * 192 + 64:pair * 192 + 192]
                nc.tensor.matmul(
                    av0[:, 0:qw], v0, et[0:jsz, 0:qw],
                    start=(jc == 0), stop=(jc == 4), skip_group_check=True)
                nc.tensor.matmul(
                    av1[:, 0:qw], v1, et[0:jsz, NA:NA + qw],
                    start=(jc == 0), stop=(jc == 4), skip_group_check=True)

            def emit_norm(ph):
                # Copy each av PSUM bank to SBUF on the scalar engine right
                # away (releases the bank for the next phase), then run the
                # whole reciprocal/normalize chain on the SBUF copy.
                # reciprocal_approx_fast only works on partitions 0:64 (custom
                # DVE ucode), so both halves run it there.
                q0, qw = waves[ph][1], waves[ph][2]
                av0, av1 = avs.pop(ph)
                avc0 = sb.tile([P, NA], f32, tag="avc", bufs=4,
                               name=f"avc{b}_{pair}_{ph}_0")
                avc1 = sb.tile([P, NA], f32, tag="avc", bufs=4,
                               name=f"avc{b}_{pair}_{ph}_1")
                nc.scalar.copy(avc0[:, 0:qw], av0[:, 0:qw])
                nc.scalar.copy(avc1[:, 0:qw], av1[:, 0:qw])
                # half 0: num rows 0:64, den rows 64:128 -> move den down
                rec0 = sb.tile([P, 2 * NA], f32, tag="rec", bufs=4,
                               name=f"rec{b}_{pair}_{ph}_0")
                nc.gpsimd.dma_start(rec0[0:64, 0:qw], avc0[64:128, 0:qw])
                nc.vector.reciprocal_approx_fast(
                    out=rec0[0:64, NA:NA + qw], in_=rec0[0:64, 0:qw])
                nc.vector.tensor_tensor(
                    no_t[0:64, q0:q0 + qw], avc0[0:64, 0:qw],
                    rec0[0:64, NA:NA + qw], OP.mult)
                # half 1: den rows 0:64, num rows 64:128 -> move rec up
                rec1 = sb.tile([P, 2 * NA], f32, tag="rec", bufs=4,
                               name=f"rec{b}_{pair}_{ph}_1")
                nc.vector.reciprocal_approx_fast(
                    out=rec1[0:64, 0:qw], in_=avc1[0:64, 0:qw])
                nc.gpsimd.dma_start(rec1[64:128, 0:qw], rec1[0:64, 0:qw])
                nc.vector.tensor_tensor(
                    no_t[64:128, q0:q0 + qw], avc1[64:128, 0:qw],
                    rec1[64:128, 0:qw], OP.mult)

            for idx, (ph, q0, qw, jc) in enumerate(wl):
                js, jsz = TOK[jc]
                if jc == 0:
                    avs[ph] = (
                        avp.tile([P, 512], f32, tag="av",
                                 name=f"av{b}_{pair}_{ph}_0"),
                        avp.tile([P, 512], f32, tag="av",
                                 name=f"av{b}_{pair}_{ph}_1"))
                sct = scp.tile([P, 1024], f32, tag="sc",
                               name=f"sc{b}_{pair}_{ph}_{jc}")
                # both heads' score matmuls back-to-back: K=64 row groups
                # (0:64 / 64:128) overlap on the PE
                nc.tensor.matmul(
                    sct[0:jsz, 0:qw], kt[0:64, js:js + jsz],
                    qt[0:64, q0:q0 + qw], skip_group_check=True)
                nc.tensor.matmul(
                    sct[0:jsz, 512:512 + qw], kt[64:128, js:js + jsz],
                    qt[64:128, q0:q0 + qw], skip_group_check=True)
                et = sb.tile([P, 2 * NA], bf16, tag="e", bufs=8,
                             name=f"e{b}_{pair}_{ph}_{jc}")
                nc.scalar.activation(
                    et[0:jsz].rearrange("p (a q) -> p a q", a=2)[:, :, 0:qw],
                    sct[0:jsz].rearrange("p (a q) -> p a q", a=2)[:, :, 0:qw],
                    AF.Exp, scale=0.125)
                ets[idx] = et
                if idx == 0 and carry[0] is not None:
                    # previous pair's last AVs + norm run behind our first
                    # wave so its exp latency is hidden
                    carry[0]()
                    carry[0] = None
                # AV lags the score wave by two steps: a packed wave is
                # shorter than one exp, so lag-1 would stall on ACT
                if idx > 1:
                    emit_av(idx - 2)
                    if wl[idx - 2][3] == 4:  # that wave closed a phase
                        emit_norm(wl[idx - 2][0])
            no_sb.append(no_t)

            def finish():
                emit_av(len(wl) - 2)
                emit_av(len(wl) - 1)
                emit_norm(wl[-1][0])
            return finish

        def emit_proj(b, no_sb):
            # transposed projection: out[c_out, tokens] -- full 128-row
            # c-tiles (no 65-token partition waste), free dim 577 not 768,
            # and the bias becomes per-partition so it rides the scalar
            # engine's activation.  The host untransposes [C, N] -> [N, C].
            for ct in range(KT):
                ot = sb.tile([P, N], bf16, tag="outp", bufs=4, name=f"o{b}_{ct}")
                ppA = ps.tile([P, 512], f32, tag="ps", name=f"ppA{b}_{ct}")
                ppB = ps.tile([P, 512], f32, tag="ps", name=f"ppB{b}_{ct}")
                for kk in range(KT):
                    nc.tensor.matmul(
                        ppA[:, 0:NA], wp_sb[kk][:, ct * P:(ct + 1) * P],
                        no_sb[kk][:, 0:NA],
                        start=(kk == 0), stop=(kk == KT - 1))
                for kk in range(KT):
                    nc.tensor.matmul(
                        ppB[:, 0:NBW], wp_sb[kk][:, ct * P:(ct + 1) * P],
                        no_sb[kk][:, NA:N],
                        start=(kk == 0), stop=(kk == KT - 1))
                nc.vector.tensor_scalar(
                    ot[:, 0:NA], ppA[:, 0:NA], bpB[:, ct:ct + 1],
                    None, OP.add)
                nc.vector.tensor_scalar(
                    ot[:, NA:N], ppB[:, 0:NBW], bpB[:, ct:ct + 1],
                    None, OP.add)
                nc.sync.dma_start(out_d[b, ct * P:(ct + 1) * P, :], ot)

        pending = []
        state = {}

        def flush_carry():
            if carry[0] is not None:
                carry[0]()
                carry[0] = None
                carry_b[0] = None

        def pop_unit():
            if not pending:
                return
            pb, pr = pending.pop(0)
            st = state[pb]
            fin = emit_attention(pb, pr, st["qc"], st["vts"], st["no_sb"])

            def full_fin(pb=pb, pr=pr, fin=fin, st=st):
                fin()
                if pr == NPAIR - 1:
                    emit_proj(pb, st["no_sb"])

            carry[0] = full_fin
            carry_b[0] = pb

        xts_next = xts0
        for b in range(NB):
            xts = xts_next
            ut_all = sb.tile([P, 12, N], bf16, tag="ut", bufs=1, name=f"ut{b}")
            us_all = sb.tile([P, 12, N], bf16, tag="us", bufs=1, name=f"us{b}")
            qc_all = sb.tile([P, 12, N], bf16, tag="qc", bufs=2, name=f"qc{b}")
            drains = []
            if b == 0:
                drains = emit_qk_warmup(xts, ut_all, qc_all)
                m_start = 4
            else:
                m_start = 0
            state[b] = dict(vts=None, qc=qc_all, no_sb=[])
            for m in range(m_start, 12):
                if b == 0 and m % 2 == 1:
                    # attention hasn't started yet in batch 0: borrow the
                    # idle av banks so the qk chain isn't serialized on the
                    # two ps banks' DVE drains
                    emit_qk_tile(b, m, xts, ut_all, qc_all, avp, "av")
                else:
                    emit_qk_tile(b, m, xts, ut_all, qc_all)
                if drains:
                    drains.pop(0)()   # deferred warmup drains, spread out
                if m % 2 == 1:
                    pop_unit()
                if m == 5:
                    while drains:     # swap needs all of ut[:, 0:6]
                        drains.pop(0)()
                    emit_swap(b, 0, ut_all, us_all)
                    nc.vector.tensor_tensor(
                        qc_all[:, 0:6, :], qc_all[:, 0:6, :],
                        us_all[:, 0:6, :], OP.add)
                if m == 8 and b + 1 < NB:
                    xts_next = emit_x(b + 1)
            emit_swap(b, 1, ut_all, us_all)
            state[b]["vts"] = emit_front(b, xts)
            if b == 0:
                emit_proj_consts()
            for pair in range(NPAIR):
                nc.vector.tensor_tensor(
                    qc_all[:, 6 + pair, :], qc_all[:, 6 + pair, :],
                    us_all[:, 6 + pair, :], OP.add)
                pending.append((b, pair))
        while pending:
            pop_unit()
        flush_carry()

    nc.compile()
    return nc


def _get_nc():
    if "nc" not in _cache:
        _cache["nc"] = _build()
    return _cache["nc"]


def _prep_shared(inputs):
    """Host-side layout prep shared across cores (numpy only)."""
    import ml_dtypes

    bf = ml_dtypes.bfloat16
    w_qkv = np.ascontiguousarray(np.asarray(inputs["w_qkv"], np.float32)).astype(bf)
    w_proj = np.ascontiguousarray(np.asarray(inputs["w_proj"], np.float32)).astype(bf)
    b_qkv = np.asarray(inputs["b_qkv"], np.float32)
    b_proj = np.asarray(inputs["b_proj"], np.float32)
    sin = np.asarray(inputs["rope_sin"], np.float32)  # [576, 64]
    cos = np.asarray(inputs["rope_cos"], np.float32)

    bqk2 = np.ascontiguousarray(b_qkv.reshape(18, P).T)          # [128, 18]
    # fold the v bias through the projection: out = (attn+b_v) W_p + b_p
    # = attn W_p + (b_v W_p + b_p), since softmax weights sum to 1
    wp32 = np.asarray(inputs["w_proj"], np.float32)
    bp_eff = b_qkv[2 * C:] @ wp32 + b_proj
    bpb = np.ascontiguousarray(bp_eff.reshape(KT, P).T)       # [128, 6]

    cost = np.ones((64, N), np.float32)
    cost[:, 1:] = cos.T
    # sinS holds sin at the swapped index with the rotate-half sign pattern:
    # rows 0:32 <- +sin cols 32:64 ; rows 32:64 <- -sin cols 0:32
    sins = np.zeros((64, N), np.float32)
    sins[0:32, 1:] = sin.T[32:64]
    sins[32:64, 1:] = -sin.T[0:32]

    return {
        "w_qkv": w_qkv,
        "w_proj": w_proj,
        "bqk2": bqk2.astype(np.float32),
        "bpb": bpb.astype(np.float32),
        "cost": cost.astype(bf),
        "sins": sins.astype(bf),
    }


last_results = None


def kernel(**inputs):
    global last_results
    import ml_dtypes

    from concourse.bass_utils import run_bass_kernel_spmd

    nc = _get_nc()
    bf = ml_dtypes.bfloat16
    x = np.asarray(inputs["x"], np.float32)
    # host-side transpose + bf16 cast: [B, N, C] -> [B, C, N]
    xt_all = np.ascontiguousarray(x.transpose(0, 2, 1)).astype(bf)
    shared = _prep_shared(inputs)

    in_maps = []
    for c in range(NCORES):
        m = dict(shared)
        m["xt"] = np.ascontiguousarray(xt_all[c * NB:(c + 1) * NB])
        in_maps.append(m)

    res = run_bass_kernel_spmd(nc, in_maps, core_ids=list(range(NCORES)))
    last_results = res
    full = np.concatenate(
        [np.asarray(res.results[c]["out"], np.float32) for c in range(NCORES)],
        axis=0)                                   # [B, C, N]
    return np.ascontiguousarray(full.transpose(0, 2, 1))
